# revision 16
# baseline (speedup 1.0000x reference)
"""Trainium2 Bass kernel for the counting-criterion loss.

Computes, for output/density_map of shape [32, 1, 512, 512] and bboxes [32, 3, 4]:
  dmap_loss  = sum((output - density_map)^2) / num_objects
  count_loss = mean_b((sum(output_b) - sum(density_map_b))^2)
  min_count  = sum_boxes(relu(1 - box_sum))   with box sums over [y1:y2, x1:x2)

Strategy: data-parallel over the batch — core i handles images [4i, 4i+4).
On each core, per image:
  - one DVE scalar_tensor_tensor gives diff = o - d plus per-partition sum(diff)
  - one ACT Square activation with accum_out gives per-partition sum(diff^2)
  - box sums via PE: for each x-chunk, O_chunk^T (stationary [128,128]) @
    rowmask (moving [128,3]) accumulated over the 4 y-chunks -> psum[x, (c,j)];
    multiply by the column mask on DVE, then reduce over the x partitions with
    a matmul against a ones column -> psum2[(c,j), 1] (partition-major).

Tail latency tricks (the DMA stream is the roofline; everything else hides
behind it except the last handoff):
  - the LAST image loads all of o first, then d in shrinking pieces
    (quarter, quarter, quarter, eighth, sixteenth, sixteenth) so the only
    work after the final input byte is one [128,128] stt + square on DVE;
  - the final accumulator write-back to DRAM is a SWDGE kv_writeback
    prepared early (descriptor gen on the idle Pool engine) and fired with
    trigger_dma after the last accum lands — skipping the HWDGE (625ns) and
    DGE (650ns) issue latencies of a normal DMA.
Final tiny reductions (cross-partition sums, relu, squares, weights) run on
the host from each core's [128, 25] accumulator.
"""

import numpy as np
from contextlib import ExitStack

import concourse.bass as bass
import concourse.mybir as mybir
import concourse.tile as tile
from concourse import bacc
from concourse.bass_utils import run_bass_kernel_spmd

N_CORES = 8
B, H, W = 32, 512, 512
NIMG = B // N_CORES  # images per core
P = 128              # SBUF partitions
NCH = H // P         # row chunks per image (and col chunks: W//P)
NB = 3               # boxes per image
F32 = mybir.dt.float32

# accumulator columns: diff sums [0:NCOL), squared sums [NCOL:2*NCOL),
# box partials in column 2*NCOL (rows 32*img + cx*NB + j)
NCOL = 2 * (NIMG - 1) + 7   # imgs 0..2 as halves, img 3 as 7 d-pieces
ACCW = 2 * NCOL + 1

_PROG = None


def _build_program():
    nc = bacc.Bacc(
        "TRN2",
        target_bir_lowering=False,
        debug=False,
        num_devices=N_CORES,
    )
    o_d = nc.dram_tensor("o", [NIMG, H, W], F32, kind="ExternalInput").ap()
    d_d = nc.dram_tensor("d", [NIMG, H, W], F32, kind="ExternalInput").ap()
    # packed masks per image: cols 0:NCH*NB row mask [y%128, (y//128, j)],
    # cols NCH*NB:2*NCH*NB col mask [x%128, (x//128, j)]
    msk_d = nc.dram_tensor(
        "msk", [NIMG, P, 2 * NCH * NB], F32, kind="ExternalInput"
    ).ap()
    acc_d = nc.dram_tensor("acc", [P, ACCW], F32, kind="ExternalOutput").ap()

    # DRAM views: image rows split as y = c*128 + p  ->  [img, p, c, x]
    o_r = o_d.rearrange("n (c p) x -> n p c x", p=P)
    d_r = d_d.rearrange("n (c p) x -> n p c x", p=P)

    with tile.TileContext(nc) as tc, ExitStack() as ctx:
        io_pool = ctx.enter_context(tc.tile_pool(name="io", bufs=2))
        qio_pool = ctx.enter_context(tc.tile_pool(name="qio", bufs=1))
        mask_pool = ctx.enter_context(tc.tile_pool(name="mask", bufs=2))
        work_pool = ctx.enter_context(tc.tile_pool(name="work", bufs=2))
        psum_pool = ctx.enter_context(tc.tile_pool(name="psum", bufs=2, space="PSUM"))
        acc_pool = ctx.enter_context(tc.tile_pool(name="acc", bufs=1))

        acc = acc_pool.tile([P, ACCW], F32)
        nc.vector.memset(acc[:], 0.0)
        ones_t = acc_pool.tile([P, 1], F32)
        nc.vector.memset(ones_t[:], 1.0)
        # box partials accumulate at partition (32*img + cx*NB + j); matmul
        # out base partitions must be 0/32/64, so image 3 gets its own tile
        ps2 = psum_pool.tile([96, 1], F32, tag="ps2")
        ps2b = psum_pool.tile([NCH * NB, 1], F32, tag="ps2b")


        def box_work(img, o_chunks, msk_t):
            """o_chunks: list of (tile, free-index) giving [128, 512] y-chunk APs."""
            ps = psum_pool.tile([P, NCH * NB], F32, tag="ps")
            for cx in range(NCH):
                for cy in range(NCH):
                    t, idx = o_chunks[cy]
                    nc.tensor.matmul(
                        ps[:, cx * NB : (cx + 1) * NB],
                        lhsT=t[:, idx, cx * P : (cx + 1) * P],
                        rhs=msk_t[:, cy * NB : (cy + 1) * NB],
                        start=(cy == 0),
                        stop=(cy == NCH - 1),
                    )
            masked_t = work_pool.tile([P, NCH * NB], F32, tag="masked")
            nc.vector.tensor_tensor(
                out=masked_t[:],
                in0=ps[:],
                in1=msk_t[:, NCH * NB : 2 * NCH * NB],
                op=mybir.AluOpType.mult,
            )
            # reduce over the 128 x-partitions; output lands partition-major
            # so the whole accumulator stays narrow: ps2[(cx,j), 0]
            out_ap = (
                ps2[32 * img : 32 * img + NCH * NB, :] if img < 3 else ps2b[:]
            )
            nc.tensor.matmul(
                out_ap, lhsT=masked_t[:], rhs=ones_t[:], start=True, stop=True
            )

        def diff_work(o_ap, d_ap, col, square_on_dve=False, tag=""):
            """stt diff + square over one chunk, accumulating into column col.

            The square runs on ACT by default (hides under DMA); for the tail
            chunk it runs on DVE so the critical chain stays on one engine.
            """
            diff_t = work_pool.tile(
                list(o_ap.shape), F32, tag="diff" + tag, bufs=5 if tag else None
            )
            nc.vector.scalar_tensor_tensor(
                out=diff_t[:],
                in0=o_ap,
                scalar=0.0,
                in1=d_ap,
                op0=mybir.AluOpType.bypass,
                op1=mybir.AluOpType.subtract,
                accum_out=acc[:, col : col + 1],
            )
            sq_t = work_pool.tile(
                list(o_ap.shape), F32, tag="sq" + tag, bufs=5 if tag else None
            )
            if square_on_dve:
                nc.vector.scalar_tensor_tensor(
                    out=sq_t[:],
                    in0=diff_t[:],
                    scalar=0.0,
                    in1=diff_t[:],
                    op0=mybir.AluOpType.bypass,
                    op1=mybir.AluOpType.mult,
                    accum_out=acc[:, NCOL + col : NCOL + col + 1],
                )
            else:
                nc.scalar.activation(
                    sq_t[:],
                    diff_t[:],
                    mybir.ActivationFunctionType.Square,
                    accum_out=acc[:, NCOL + col : NCOL + col + 1],
                )

        msk_all = mask_pool.tile([P, NIMG, 2 * NCH * NB], F32)

        # images 0..NIMG-2: half-image pipeline (keeps DVE/ACT streaming
        # steadily behind the DMA instead of big 2.2us blocks)
        HC = NCH // 2
        for img in range(NIMG - 1):
            halves = []
            for h in range(2):
                o_t = io_pool.tile([P, HC, W], F32, tag=f"o{h}")
                nc.sync.dma_start(o_t[:], o_r[img, :, h * HC : (h + 1) * HC])
                d_t = io_pool.tile([P, HC, W], F32, tag=f"d{h}")
                nc.sync.dma_start(d_t[:], d_r[img, :, h * HC : (h + 1) * HC])
                if img == 0 and h == 0:
                    # all masks in one small DMA, tucked behind the first pair
                    nc.sync.dma_start(
                        msk_all[:], msk_d.rearrange("n p m -> p n m")
                    )
                diff_work(o_t[:], d_t[:], 2 * img + h)
                halves.append(o_t)
            box_work(
                img,
                [(halves[c // HC], c % HC) for c in range(NCH)],
                msk_all[:, img],
            )

        # last image: ALL of o first (its box work then completes early),
        # then d in shrinking pieces so the post-stream tail is only a
        # [128,128] sixteenth's diff+square.
        img = NIMG - 1
        o_halves = []
        for h in range(2):
            o_t = qio_pool.tile([P, HC, W], F32, tag=f"o3h{h}")
            nc.sync.dma_start(o_t[:], o_r[img, :, h * HC : (h + 1) * HC])
            o_halves.append(o_t)
        box_work(
            img,
            [(o_halves[c // HC], c % HC) for c in range(NCH)],
            msk_all[:, img],
        )

        # copy box partials into the accumulator now — all box inputs are
        # o/mask only, so this stays off the tail and ahead of the piece
        # stts in the DVE queue
        for i in range(NIMG):
            src_ap = ps2[32 * i : 32 * i + NCH * NB, :] if i < 3 else ps2b[:]
            nc.vector.tensor_copy(
                acc[32 * i : 32 * i + NCH * NB, 2 * NCOL : 2 * NCOL + 1],
                src_ap,
            )

        # d pieces: (chunk, x0, x1) — three quarters, then four sixteenths so
        # the post-stream tail is only a [128,128] diff+square
        S16 = W // 4
        pieces = [(0, 0, W), (1, 0, W), (2, 0, W)] + [
            (3, k * S16, (k + 1) * S16) for k in range(4)
        ]
        d_tiles = {}
        for c in range(NCH):
            d_tiles[c] = dq_t = qio_pool.tile(
                [P, 1, W], F32, tag=f"d3c{c}", name=f"d3c{c}"
            )
        for i, (c, x0, x1) in enumerate(pieces):
            dq = d_tiles[c]
            nc.sync.dma_start(dq[:, 0, x0:x1], d_r[img, :, c, x0:x1])
            o_ap = o_halves[c // HC][:, c % HC, x0:x1]
            diff_work(
                o_ap,
                dq[:, 0, x0:x1],
                2 * (NIMG - 1) + i,
                square_on_dve=(i == len(pieces) - 1),
                tag="q",
            )

        # final writeback from the Pool queue: desc-gen on the idle Pool
        # engine (no HWDGE) is ~260ns cheaper than the SP issue path
        nc.gpsimd.dma_start(acc_d, acc[:])

    nc.compile()
    return nc


def _get_program():
    global _PROG
    if _PROG is None:
        _PROG = _build_program()
    return _PROG


def _prep_inputs(output, density_map, bboxes):
    o = np.ascontiguousarray(np.asarray(output, dtype=np.float32).reshape(B, H, W))
    dm = np.ascontiguousarray(
        np.asarray(density_map, dtype=np.float32).reshape(B, H, W)
    )
    bb = np.clip(np.asarray(bboxes).astype(np.int64), 0, W).astype(np.int32)
    x1, y1, x2, y2 = bb[..., 0], bb[..., 1], bb[..., 2], bb[..., 3]
    x2 = np.maximum(x2, x1)
    y2 = np.maximum(y2, y1)

    ar = np.arange(H, dtype=np.int32)
    # rm[b, y, j] = 1 if y1 <= y < y2, laid out as [b, y%128, (y//128, j)]
    rm = (
        (ar[None, :, None] >= y1[:, None, :]) & (ar[None, :, None] < y2[:, None, :])
    ).astype(np.float32)
    rm = rm.reshape(B, NCH, P, NB).transpose(0, 2, 1, 3).reshape(B, P, NCH * NB)
    # cm[b, j, x] = 1 if x1 <= x < x2, laid out as [b, x%128, (x//128, j)]
    cm = (
        (ar[None, None, :] >= x1[:, :, None]) & (ar[None, None, :] < x2[:, :, None])
    ).astype(np.float32)
    cm = cm.reshape(B, NB, NCH, P).transpose(0, 3, 2, 1).reshape(B, P, NCH * NB)
    msk = np.ascontiguousarray(np.concatenate([rm, cm], axis=2))  # [B, P, 24]
    return o, dm, msk


def kernel(output, density_map, bboxes, num_objects):
    o, dm, msk = _prep_inputs(output, density_map, bboxes)

    nc = _get_program()
    in_maps = [
        {
            "o": o[i * NIMG : (i + 1) * NIMG],
            "d": dm[i * NIMG : (i + 1) * NIMG],
            "msk": msk[i * NIMG : (i + 1) * NIMG],
        }
        for i in range(N_CORES)
    ]
    res = run_bass_kernel_spmd(nc, in_maps, core_ids=list(range(N_CORES)))

    def _per_img(cols):
        # columns: img0..img2 as 2 halves each, img3 as its 7 d-pieces
        firsts = [cols[2 * i] + cols[2 * i + 1] for i in range(NIMG - 1)]
        return np.array(firsts + [cols[2 * (NIMG - 1) :].sum()])

    accs = [r["acc"] for r in res.results]  # [128, ACCW] each
    per_img_d = np.concatenate(
        [_per_img(a[:, :NCOL].sum(axis=0, dtype=np.float64)) for a in accs]
    )  # [B] sum(o - d) per image
    sq_total = float(
        sum(a[:, NCOL : 2 * NCOL].sum(dtype=np.float64) for a in accs)
    )  # sum((o - d)^2)
    # acc[32*img + (cx, j), 2*NCOL] -> sum over cx -> [NIMG, NB], image-major
    box_sums = np.concatenate(
        [
            a[:, 2 * NCOL]
            .reshape(NIMG, 32)[:, : NCH * NB]
            .reshape(NIMG, NCH, NB)
            .sum(axis=1, dtype=np.float64)
            .reshape(-1)
            for a in accs
        ]
    )  # [B*NB]

    dmap_loss = sq_total / float(num_objects)
    count_loss = float(np.mean(per_img_d**2))
    min_count = float(np.maximum(0.0, 1.0 - box_sums).sum())
    return np.array([dmap_loss, count_loss, min_count], dtype=np.float32)


# revision 17
# speedup vs baseline: 1.0299x; 1.0299x over previous
"""Trainium2 Bass kernel for the counting-criterion loss.

Computes, for output/density_map of shape [32, 1, 512, 512] and bboxes [32, 3, 4]:
  dmap_loss  = sum((output - density_map)^2) / num_objects
  count_loss = mean_b((sum(output_b) - sum(density_map_b))^2)
  min_count  = sum_boxes(relu(1 - box_sum))   with box sums over [y1:y2, x1:x2)

Strategy: data-parallel over the batch — core i handles images [4i, 4i+4).
On each core, per image:
  - one DVE scalar_tensor_tensor gives diff = o - d plus per-partition sum(diff)
  - one ACT Square activation with accum_out gives per-partition sum(diff^2)
  - box sums via PE: for each x-chunk, O_chunk^T (stationary [128,128]) @
    rowmask (moving [128,3]) accumulated over the 4 y-chunks -> psum[x, (c,j)];
    multiply by the column mask on DVE, then reduce over the x partitions with
    a matmul against a ones column -> psum2[(c,j), 1] (partition-major).

Tail latency tricks (the DMA stream is the roofline; everything else hides
behind it except the last handoff):
  - the LAST image loads all of o first, then d in shrinking pieces
    (quarter, quarter, quarter, eighth, sixteenth, sixteenth) so the only
    work after the final input byte is one [128,128] stt + square on DVE;
  - the final accumulator write-back to DRAM is a SWDGE kv_writeback
    prepared early (descriptor gen on the idle Pool engine) and fired with
    trigger_dma after the last accum lands — skipping the HWDGE (625ns) and
    DGE (650ns) issue latencies of a normal DMA.
Final tiny reductions (cross-partition sums, relu, squares, weights) run on
the host from each core's [128, 25] accumulator.
"""

import numpy as np
from contextlib import ExitStack

import concourse.bass as bass
import concourse.mybir as mybir
import concourse.tile as tile
from concourse import bacc
from concourse.bass_utils import run_bass_kernel_spmd

N_CORES = 8
B, H, W = 32, 512, 512
NIMG = B // N_CORES  # images per core
P = 128              # SBUF partitions
NCH = H // P         # row chunks per image (and col chunks: W//P)
NB = 3               # boxes per image
F32 = mybir.dt.float32

# accumulator columns: diff sums [0:NCOL), squared sums [NCOL:2*NCOL),
# box partials in column 2*NCOL (rows 32*img + cx*NB + j)
NCOL = 2 * (NIMG - 1) + 5   # imgs 0..2 as halves, img 3 as 5 d-pieces
ACCW = 2 * NCOL + 1

_PROG = None


def _build_program():
    nc = bacc.Bacc(
        "TRN2",
        target_bir_lowering=False,
        debug=False,
        num_devices=N_CORES,
    )
    o_d = nc.dram_tensor("o", [NIMG, H, W], F32, kind="ExternalInput").ap()
    d_d = nc.dram_tensor("d", [NIMG, H, W], F32, kind="ExternalInput").ap()
    # packed masks per image: cols 0:NCH*NB row mask [y%128, (y//128, j)],
    # cols NCH*NB:2*NCH*NB col mask [x%128, (x//128, j)]
    msk_d = nc.dram_tensor(
        "msk", [NIMG, P, 2 * NCH * NB], F32, kind="ExternalInput"
    ).ap()
    acc_d = nc.dram_tensor("acc", [P, ACCW], F32, kind="ExternalOutput").ap()

    # DRAM views: image rows split as y = c*128 + p  ->  [img, p, c, x]
    o_r = o_d.rearrange("n (c p) x -> n p c x", p=P)
    d_r = d_d.rearrange("n (c p) x -> n p c x", p=P)

    with tile.TileContext(nc) as tc, ExitStack() as ctx:
        io_pool = ctx.enter_context(tc.tile_pool(name="io", bufs=2))
        qio_pool = ctx.enter_context(tc.tile_pool(name="qio", bufs=1))
        mask_pool = ctx.enter_context(tc.tile_pool(name="mask", bufs=2))
        work_pool = ctx.enter_context(tc.tile_pool(name="work", bufs=2))
        psum_pool = ctx.enter_context(tc.tile_pool(name="psum", bufs=2, space="PSUM"))
        acc_pool = ctx.enter_context(tc.tile_pool(name="acc", bufs=1))

        acc = acc_pool.tile([P, ACCW], F32)
        nc.vector.memset(acc[:], 0.0)
        ones_t = acc_pool.tile([P, 1], F32)
        nc.vector.memset(ones_t[:], 1.0)
        # box partials accumulate at partition (32*img + cx*NB + j); matmul
        # out base partitions must be 0/32/64, so image 3 gets its own tile
        ps2 = psum_pool.tile([96, 1], F32, tag="ps2")
        ps2b = psum_pool.tile([NCH * NB, 1], F32, tag="ps2b")


        def box_work(img, o_chunks, msk_t):
            """o_chunks: list of (tile, free-index) giving [128, 512] y-chunk APs."""
            ps = psum_pool.tile([P, NCH * NB], F32, tag="ps")
            for cx in range(NCH):
                for cy in range(NCH):
                    t, idx = o_chunks[cy]
                    nc.tensor.matmul(
                        ps[:, cx * NB : (cx + 1) * NB],
                        lhsT=t[:, idx, cx * P : (cx + 1) * P],
                        rhs=msk_t[:, cy * NB : (cy + 1) * NB],
                        start=(cy == 0),
                        stop=(cy == NCH - 1),
                    )
            masked_t = work_pool.tile([P, NCH * NB], F32, tag="masked")
            nc.vector.tensor_tensor(
                out=masked_t[:],
                in0=ps[:],
                in1=msk_t[:, NCH * NB : 2 * NCH * NB],
                op=mybir.AluOpType.mult,
            )
            # reduce over the 128 x-partitions; output lands partition-major
            # so the whole accumulator stays narrow: ps2[(cx,j), 0]
            out_ap = (
                ps2[32 * img : 32 * img + NCH * NB, :] if img < 3 else ps2b[:]
            )
            nc.tensor.matmul(
                out_ap, lhsT=masked_t[:], rhs=ones_t[:], start=True, stop=True
            )

        def diff_work(o_ap, d_ap, col, square_on_dve=False, tag=""):
            """stt diff + square over one chunk, accumulating into column col.

            The square runs on ACT by default (hides under DMA); for the tail
            chunk it runs on DVE so the critical chain stays on one engine.
            """
            diff_t = work_pool.tile(
                list(o_ap.shape), F32, tag="diff" + tag, bufs=5 if tag else None
            )
            nc.vector.scalar_tensor_tensor(
                out=diff_t[:],
                in0=o_ap,
                scalar=0.0,
                in1=d_ap,
                op0=mybir.AluOpType.bypass,
                op1=mybir.AluOpType.subtract,
                accum_out=acc[:, col : col + 1],
            )
            sq_t = work_pool.tile(
                list(o_ap.shape), F32, tag="sq" + tag, bufs=5 if tag else None
            )
            if square_on_dve:
                nc.vector.scalar_tensor_tensor(
                    out=sq_t[:],
                    in0=diff_t[:],
                    scalar=0.0,
                    in1=diff_t[:],
                    op0=mybir.AluOpType.bypass,
                    op1=mybir.AluOpType.mult,
                    accum_out=acc[:, NCOL + col : NCOL + col + 1],
                )
            else:
                nc.scalar.activation(
                    sq_t[:],
                    diff_t[:],
                    mybir.ActivationFunctionType.Square,
                    accum_out=acc[:, NCOL + col : NCOL + col + 1],
                )

        msk_all = mask_pool.tile([P, NIMG, 2 * NCH * NB], F32)

        # images 0..NIMG-2: half-image pipeline (keeps DVE/ACT streaming
        # steadily behind the DMA instead of big 2.2us blocks)
        HC = NCH // 2
        for img in range(NIMG - 1):
            halves = []
            for h in range(2):
                o_t = io_pool.tile([P, HC, W], F32, tag=f"o{h}")
                nc.sync.dma_start(o_t[:], o_r[img, :, h * HC : (h + 1) * HC])
                d_t = io_pool.tile([P, HC, W], F32, tag=f"d{h}")
                nc.sync.dma_start(d_t[:], d_r[img, :, h * HC : (h + 1) * HC])
                if img == 0 and h == 0:
                    # all masks in one small DMA, tucked behind the first pair
                    nc.sync.dma_start(
                        msk_all[:], msk_d.rearrange("n p m -> p n m")
                    )
                diff_work(o_t[:], d_t[:], 2 * img + h)
                halves.append(o_t)
            box_work(
                img,
                [(halves[c // HC], c % HC) for c in range(NCH)],
                msk_all[:, img],
            )

        # last image: interleave o/d for engine pacing, but finish o (and
        # the box work) before the final d pieces so the post-stream tail is
        # only a small diff+square chain plus the prepared-early writeback
        img = NIMG - 1
        o_halves = []
        d_tiles = {}
        for c in range(NCH):
            d_tiles[c] = dq_t = qio_pool.tile(
                [P, 1, W], F32, tag=f"d3c{c}", name=f"d3c{c}"
            )

        def d_piece(i, c, x0, x1, on_dve=False):
            dq = d_tiles[c]
            nc.sync.dma_start(dq[:, 0, x0:x1], d_r[img, :, c, x0:x1])
            diff_work(
                o_halves[c // HC][:, c % HC, x0:x1],
                dq[:, 0, x0:x1],
                2 * (NIMG - 1) + i,
                square_on_dve=on_dve,
                tag="q",
            )

        o_t0 = qio_pool.tile([P, HC, W], F32, tag="o3h0")
        nc.sync.dma_start(o_t0[:], o_r[img, :, 0:HC])
        o_halves.append(o_t0)
        d_piece(0, 0, 0, W)
        o_t1 = qio_pool.tile([P, HC, W], F32, tag="o3h1")
        nc.sync.dma_start(o_t1[:], o_r[img, :, HC : 2 * HC])
        o_halves.append(o_t1)
        d_piece(1, 1, 0, W)

        # all of o is in flight: box work + accumulator copies leave the
        # tail before the last d pieces even arrive
        box_work(
            img,
            [(o_halves[c // HC], c % HC) for c in range(NCH)],
            msk_all[:, img],
        )
        for i in range(NIMG):
            src_ap = ps2[32 * i : 32 * i + NCH * NB, :] if i < 3 else ps2b[:]
            nc.vector.tensor_copy(
                acc[32 * i : 32 * i + NCH * NB, 2 * NCOL : 2 * NCOL + 1],
                src_ap,
            )

        d_piece(2, 2, 0, W)
        d_piece(3, 3, 0, W // 2)
        d_piece(4, 3, W // 2, W, on_dve=True)

        # final writeback from the Pool queue: desc-gen on the idle Pool
        # engine (no HWDGE) is ~260ns cheaper than the SP issue path
        nc.gpsimd.dma_start(acc_d, acc[:])

    nc.compile()
    return nc


def _get_program():
    global _PROG
    if _PROG is None:
        _PROG = _build_program()
    return _PROG


def _prep_inputs(output, density_map, bboxes):
    o = np.ascontiguousarray(np.asarray(output, dtype=np.float32).reshape(B, H, W))
    dm = np.ascontiguousarray(
        np.asarray(density_map, dtype=np.float32).reshape(B, H, W)
    )
    bb = np.clip(np.asarray(bboxes).astype(np.int64), 0, W).astype(np.int32)
    x1, y1, x2, y2 = bb[..., 0], bb[..., 1], bb[..., 2], bb[..., 3]
    x2 = np.maximum(x2, x1)
    y2 = np.maximum(y2, y1)

    ar = np.arange(H, dtype=np.int32)
    # rm[b, y, j] = 1 if y1 <= y < y2, laid out as [b, y%128, (y//128, j)]
    rm = (
        (ar[None, :, None] >= y1[:, None, :]) & (ar[None, :, None] < y2[:, None, :])
    ).astype(np.float32)
    rm = rm.reshape(B, NCH, P, NB).transpose(0, 2, 1, 3).reshape(B, P, NCH * NB)
    # cm[b, j, x] = 1 if x1 <= x < x2, laid out as [b, x%128, (x//128, j)]
    cm = (
        (ar[None, None, :] >= x1[:, :, None]) & (ar[None, None, :] < x2[:, :, None])
    ).astype(np.float32)
    cm = cm.reshape(B, NB, NCH, P).transpose(0, 3, 2, 1).reshape(B, P, NCH * NB)
    msk = np.ascontiguousarray(np.concatenate([rm, cm], axis=2))  # [B, P, 24]
    return o, dm, msk


def kernel(output, density_map, bboxes, num_objects):
    o, dm, msk = _prep_inputs(output, density_map, bboxes)

    nc = _get_program()
    in_maps = [
        {
            "o": o[i * NIMG : (i + 1) * NIMG],
            "d": dm[i * NIMG : (i + 1) * NIMG],
            "msk": msk[i * NIMG : (i + 1) * NIMG],
        }
        for i in range(N_CORES)
    ]
    res = run_bass_kernel_spmd(nc, in_maps, core_ids=list(range(N_CORES)))

    def _per_img(cols):
        # columns: img0..img2 as 2 halves each, img3 as its 5 d-pieces
        firsts = [cols[2 * i] + cols[2 * i + 1] for i in range(NIMG - 1)]
        return np.array(firsts + [cols[2 * (NIMG - 1) :].sum()])

    accs = [r["acc"] for r in res.results]  # [128, ACCW] each
    per_img_d = np.concatenate(
        [_per_img(a[:, :NCOL].sum(axis=0, dtype=np.float64)) for a in accs]
    )  # [B] sum(o - d) per image
    sq_total = float(
        sum(a[:, NCOL : 2 * NCOL].sum(dtype=np.float64) for a in accs)
    )  # sum((o - d)^2)
    # acc[32*img + (cx, j), 2*NCOL] -> sum over cx -> [NIMG, NB], image-major
    box_sums = np.concatenate(
        [
            a[:, 2 * NCOL]
            .reshape(NIMG, 32)[:, : NCH * NB]
            .reshape(NIMG, NCH, NB)
            .sum(axis=1, dtype=np.float64)
            .reshape(-1)
            for a in accs
        ]
    )  # [B*NB]

    dmap_loss = sq_total / float(num_objects)
    count_loss = float(np.mean(per_img_d**2))
    min_count = float(np.maximum(0.0, 1.0 - box_sums).sum())
    return np.array([dmap_loss, count_loss, min_count], dtype=np.float32)


# revision 18
# speedup vs baseline: 1.0450x; 1.0147x over previous
"""Trainium2 Bass kernel for the counting-criterion loss.

Computes, for output/density_map of shape [32, 1, 512, 512] and bboxes [32, 3, 4]:
  dmap_loss  = sum((output - density_map)^2) / num_objects
  count_loss = mean_b((sum(output_b) - sum(density_map_b))^2)
  min_count  = sum_boxes(relu(1 - box_sum))   with box sums over [y1:y2, x1:x2)

Strategy: data-parallel over the batch — core i handles images [4i, 4i+4).
On each core, per image:
  - one DVE scalar_tensor_tensor gives diff = o - d plus per-partition sum(diff)
  - one ACT Square activation with accum_out gives per-partition sum(diff^2)
  - box sums via PE: for each x-chunk, O_chunk^T (stationary [128,128]) @
    rowmask (moving [128,3]) accumulated over the 4 y-chunks -> psum[x, (c,j)];
    multiply by the column mask on DVE, then reduce over the x partitions with
    a matmul against a ones column -> psum2[(c,j), 1] (partition-major).

Tail latency tricks (the DMA stream is the roofline; everything else hides
behind it except the last handoff):
  - the LAST image loads all of o first, then d in shrinking pieces
    (quarter, quarter, quarter, eighth, sixteenth, sixteenth) so the only
    work after the final input byte is one [128,128] stt + square on DVE;
  - the final accumulator write-back to DRAM is a SWDGE kv_writeback
    prepared early (descriptor gen on the idle Pool engine) and fired with
    trigger_dma after the last accum lands — skipping the HWDGE (625ns) and
    DGE (650ns) issue latencies of a normal DMA.
Final tiny reductions (cross-partition sums, relu, squares, weights) run on
the host from each core's [128, 25] accumulator.
"""

import numpy as np
from contextlib import ExitStack

import concourse.bass as bass
import concourse.mybir as mybir
import concourse.tile as tile
from concourse import bacc
from concourse.bass_utils import run_bass_kernel_spmd

N_CORES = 8
B, H, W = 32, 512, 512
NIMG = B // N_CORES  # images per core
P = 128              # SBUF partitions
NCH = H // P         # row chunks per image (and col chunks: W//P)
NB = 3               # boxes per image
F32 = mybir.dt.float32

# accumulator columns: diff sums [0:NCOL), squared sums [NCOL:2*NCOL),
# box partials in column 2*NCOL (rows 32*img + cx*NB + j)
NCOL = 2 * (NIMG - 1) + 6   # imgs 0..2 as halves, img 3 as 6 d-pieces
ACCW = 2 * NCOL + 1

_PROG = None


def _build_program():
    nc = bacc.Bacc(
        "TRN2",
        target_bir_lowering=False,
        debug=False,
        num_devices=N_CORES,
    )
    o_d = nc.dram_tensor("o", [NIMG, H, W], F32, kind="ExternalInput").ap()
    d_d = nc.dram_tensor("d", [NIMG, H, W], F32, kind="ExternalInput").ap()
    # packed masks per image: cols 0:NCH*NB row mask [y%128, (y//128, j)],
    # cols NCH*NB:2*NCH*NB col mask [x%128, (x//128, j)]
    msk_d = nc.dram_tensor(
        "msk", [NIMG, P, 2 * NCH * NB], F32, kind="ExternalInput"
    ).ap()
    acc_d = nc.dram_tensor("acc", [P, ACCW], F32, kind="ExternalOutput").ap()

    # DRAM views: image rows split as y = c*128 + p  ->  [img, p, c, x]
    o_r = o_d.rearrange("n (c p) x -> n p c x", p=P)
    d_r = d_d.rearrange("n (c p) x -> n p c x", p=P)

    with tile.TileContext(nc) as tc, ExitStack() as ctx:
        io_pool = ctx.enter_context(tc.tile_pool(name="io", bufs=2))
        qio_pool = ctx.enter_context(tc.tile_pool(name="qio", bufs=1))
        mask_pool = ctx.enter_context(tc.tile_pool(name="mask", bufs=2))
        work_pool = ctx.enter_context(tc.tile_pool(name="work", bufs=2))
        psum_pool = ctx.enter_context(tc.tile_pool(name="psum", bufs=2, space="PSUM"))
        acc_pool = ctx.enter_context(tc.tile_pool(name="acc", bufs=1))

        acc = acc_pool.tile([P, ACCW], F32)
        nc.vector.memset(acc[:], 0.0)
        ones_t = acc_pool.tile([P, 1], F32)
        nc.vector.memset(ones_t[:], 1.0)
        # box partials accumulate at partition (32*img + cx*NB + j); matmul
        # out base partitions must be 0/32/64, so image 3 gets its own tile
        ps2 = psum_pool.tile([96, 1], F32, tag="ps2")
        ps2b = psum_pool.tile([NCH * NB, 1], F32, tag="ps2b")


        def box_work(img, o_chunks, msk_t):
            """o_chunks: list of (tile, free-index) giving [128, 512] y-chunk APs."""
            ps = psum_pool.tile([P, NCH * NB], F32, tag="ps")
            for cx in range(NCH):
                for cy in range(NCH):
                    t, idx = o_chunks[cy]
                    nc.tensor.matmul(
                        ps[:, cx * NB : (cx + 1) * NB],
                        lhsT=t[:, idx, cx * P : (cx + 1) * P],
                        rhs=msk_t[:, cy * NB : (cy + 1) * NB],
                        start=(cy == 0),
                        stop=(cy == NCH - 1),
                    )
            masked_t = work_pool.tile([P, NCH * NB], F32, tag="masked")
            nc.vector.tensor_tensor(
                out=masked_t[:],
                in0=ps[:],
                in1=msk_t[:, NCH * NB : 2 * NCH * NB],
                op=mybir.AluOpType.mult,
            )
            # reduce over the 128 x-partitions; output lands partition-major
            # so the whole accumulator stays narrow: ps2[(cx,j), 0]
            out_ap = (
                ps2[32 * img : 32 * img + NCH * NB, :] if img < 3 else ps2b[:]
            )
            nc.tensor.matmul(
                out_ap, lhsT=masked_t[:], rhs=ones_t[:], start=True, stop=True
            )

        def diff_work(o_ap, d_ap, col, square_on_dve=False, tag=""):
            """stt diff + square over one chunk, accumulating into column col.

            The square runs on ACT by default (hides under DMA); for the tail
            chunk it runs on DVE so the critical chain stays on one engine.
            """
            diff_t = work_pool.tile(
                list(o_ap.shape), F32, tag="diff" + tag, bufs=5 if tag else None
            )
            nc.vector.scalar_tensor_tensor(
                out=diff_t[:],
                in0=o_ap,
                scalar=0.0,
                in1=d_ap,
                op0=mybir.AluOpType.bypass,
                op1=mybir.AluOpType.subtract,
                accum_out=acc[:, col : col + 1],
            )
            sq_t = work_pool.tile(
                list(o_ap.shape), F32, tag="sq" + tag, bufs=5 if tag else None
            )
            if square_on_dve:
                nc.vector.scalar_tensor_tensor(
                    out=sq_t[:],
                    in0=diff_t[:],
                    scalar=0.0,
                    in1=diff_t[:],
                    op0=mybir.AluOpType.bypass,
                    op1=mybir.AluOpType.mult,
                    accum_out=acc[:, NCOL + col : NCOL + col + 1],
                )
            else:
                nc.scalar.activation(
                    sq_t[:],
                    diff_t[:],
                    mybir.ActivationFunctionType.Square,
                    accum_out=acc[:, NCOL + col : NCOL + col + 1],
                )

        msk_all = mask_pool.tile([P, NIMG, 2 * NCH * NB], F32)

        # images 0..NIMG-2: half-image pipeline (keeps DVE/ACT streaming
        # steadily behind the DMA instead of big 2.2us blocks)
        HC = NCH // 2
        for img in range(NIMG - 1):
            halves = []
            for h in range(2):
                o_t = io_pool.tile([P, HC, W], F32, tag=f"o{h}")
                nc.sync.dma_start(o_t[:], o_r[img, :, h * HC : (h + 1) * HC])
                d_t = io_pool.tile([P, HC, W], F32, tag=f"d{h}")
                nc.sync.dma_start(d_t[:], d_r[img, :, h * HC : (h + 1) * HC])
                if img == 0 and h == 0:
                    # all masks in one small DMA, tucked behind the first pair
                    nc.sync.dma_start(
                        msk_all[:], msk_d.rearrange("n p m -> p n m")
                    )
                diff_work(o_t[:], d_t[:], 2 * img + h)
                halves.append(o_t)
            box_work(
                img,
                [(halves[c // HC], c % HC) for c in range(NCH)],
                msk_all[:, img],
            )

        # last image: interleave o/d for engine pacing, but finish o (and
        # the box work) before the final d pieces so the post-stream tail is
        # only a small diff+square chain plus the prepared-early writeback
        img = NIMG - 1
        o_halves = []
        d_tiles = {}
        for c in range(NCH):
            d_tiles[c] = dq_t = qio_pool.tile(
                [P, 1, W], F32, tag=f"d3c{c}", name=f"d3c{c}"
            )

        def d_piece(i, c, x0, x1, on_dve=False):
            dq = d_tiles[c]
            nc.sync.dma_start(dq[:, 0, x0:x1], d_r[img, :, c, x0:x1])
            diff_work(
                o_halves[c // HC][:, c % HC, x0:x1],
                dq[:, 0, x0:x1],
                2 * (NIMG - 1) + i,
                square_on_dve=on_dve,
                tag="q",
            )

        o_t0 = qio_pool.tile([P, HC, W], F32, tag="o3h0")
        nc.sync.dma_start(o_t0[:], o_r[img, :, 0:HC])
        o_halves.append(o_t0)
        d_piece(0, 0, 0, W)
        o_t1 = qio_pool.tile([P, HC, W], F32, tag="o3h1")
        nc.sync.dma_start(o_t1[:], o_r[img, :, HC : 2 * HC])
        o_halves.append(o_t1)
        d_piece(1, 1, 0, W)

        # all of o is in flight: box work + accumulator copies leave the
        # tail before the last d pieces even arrive
        box_work(
            img,
            [(o_halves[c // HC], c % HC) for c in range(NCH)],
            msk_all[:, img],
        )
        for i in range(NIMG):
            src_ap = ps2[32 * i : 32 * i + NCH * NB, :] if i < 3 else ps2b[:]
            nc.vector.tensor_copy(
                acc[32 * i : 32 * i + NCH * NB, 2 * NCOL : 2 * NCOL + 1],
                src_ap,
            )

        # chunks 2+3 as eighths, squares alternating ACT/DVE so both
        # engines drain the tail in parallel
        d_piece(2, 2, 0, W // 2)
        d_piece(3, 2, W // 2, W, on_dve=True)
        d_piece(4, 3, 0, W // 2)
        d_piece(5, 3, W // 2, W, on_dve=True)

        nc.sync.dma_start(acc_d, acc[:])

    nc.compile()
    return nc


def _get_program():
    global _PROG
    if _PROG is None:
        _PROG = _build_program()
    return _PROG


def _prep_inputs(output, density_map, bboxes):
    o = np.ascontiguousarray(np.asarray(output, dtype=np.float32).reshape(B, H, W))
    dm = np.ascontiguousarray(
        np.asarray(density_map, dtype=np.float32).reshape(B, H, W)
    )
    bb = np.clip(np.asarray(bboxes).astype(np.int64), 0, W).astype(np.int32)
    x1, y1, x2, y2 = bb[..., 0], bb[..., 1], bb[..., 2], bb[..., 3]
    x2 = np.maximum(x2, x1)
    y2 = np.maximum(y2, y1)

    ar = np.arange(H, dtype=np.int32)
    # rm[b, y, j] = 1 if y1 <= y < y2, laid out as [b, y%128, (y//128, j)]
    rm = (
        (ar[None, :, None] >= y1[:, None, :]) & (ar[None, :, None] < y2[:, None, :])
    ).astype(np.float32)
    rm = rm.reshape(B, NCH, P, NB).transpose(0, 2, 1, 3).reshape(B, P, NCH * NB)
    # cm[b, j, x] = 1 if x1 <= x < x2, laid out as [b, x%128, (x//128, j)]
    cm = (
        (ar[None, None, :] >= x1[:, :, None]) & (ar[None, None, :] < x2[:, :, None])
    ).astype(np.float32)
    cm = cm.reshape(B, NB, NCH, P).transpose(0, 3, 2, 1).reshape(B, P, NCH * NB)
    msk = np.ascontiguousarray(np.concatenate([rm, cm], axis=2))  # [B, P, 24]
    return o, dm, msk


def kernel(output, density_map, bboxes, num_objects):
    o, dm, msk = _prep_inputs(output, density_map, bboxes)

    nc = _get_program()
    in_maps = [
        {
            "o": o[i * NIMG : (i + 1) * NIMG],
            "d": dm[i * NIMG : (i + 1) * NIMG],
            "msk": msk[i * NIMG : (i + 1) * NIMG],
        }
        for i in range(N_CORES)
    ]
    res = run_bass_kernel_spmd(nc, in_maps, core_ids=list(range(N_CORES)))

    def _per_img(cols):
        # columns: img0..img2 as 2 halves each, img3 as its 6 d-pieces
        firsts = [cols[2 * i] + cols[2 * i + 1] for i in range(NIMG - 1)]
        return np.array(firsts + [cols[2 * (NIMG - 1) :].sum()])

    accs = [r["acc"] for r in res.results]  # [128, ACCW] each
    per_img_d = np.concatenate(
        [_per_img(a[:, :NCOL].sum(axis=0, dtype=np.float64)) for a in accs]
    )  # [B] sum(o - d) per image
    sq_total = float(
        sum(a[:, NCOL : 2 * NCOL].sum(dtype=np.float64) for a in accs)
    )  # sum((o - d)^2)
    # acc[32*img + (cx, j), 2*NCOL] -> sum over cx -> [NIMG, NB], image-major
    box_sums = np.concatenate(
        [
            a[:, 2 * NCOL]
            .reshape(NIMG, 32)[:, : NCH * NB]
            .reshape(NIMG, NCH, NB)
            .sum(axis=1, dtype=np.float64)
            .reshape(-1)
            for a in accs
        ]
    )  # [B*NB]

    dmap_loss = sq_total / float(num_objects)
    count_loss = float(np.mean(per_img_d**2))
    min_count = float(np.maximum(0.0, 1.0 - box_sums).sum())
    return np.array([dmap_loss, count_loss, min_count], dtype=np.float32)


# revision 19
# speedup vs baseline: 1.0469x; 1.0018x over previous
"""Trainium2 Bass kernel for the counting-criterion loss.

Computes, for output/density_map of shape [32, 1, 512, 512] and bboxes [32, 3, 4]:
  dmap_loss  = sum((output - density_map)^2) / num_objects
  count_loss = mean_b((sum(output_b) - sum(density_map_b))^2)
  min_count  = sum_boxes(relu(1 - box_sum))   with box sums over [y1:y2, x1:x2)

Strategy: data-parallel over the batch — core i handles images [4i, 4i+4).
On each core, per image:
  - one DVE scalar_tensor_tensor gives diff = o - d plus per-partition sum(diff)
  - one ACT Square activation with accum_out gives per-partition sum(diff^2)
  - box sums via PE: for each x-chunk, O_chunk^T (stationary [128,128]) @
    rowmask (moving [128,3]) accumulated over the 4 y-chunks -> psum[x, (c,j)];
    multiply by the column mask on DVE, then a ones-vector matmul reduces over
    the x partitions.
Final tiny reductions (cross-partition sums, relu, squares, weights) run on
the host from each core's [128,4]+[128,4]+[1,48] partial outputs.
"""

import numpy as np
from contextlib import ExitStack

import concourse.bass as bass
import concourse.mybir as mybir
import concourse.tile as tile
from concourse import bacc
from concourse.bass_utils import run_bass_kernel_spmd

N_CORES = 8
B, H, W = 32, 512, 512
NIMG = B // N_CORES  # images per core
P = 128              # SBUF partitions
NCH = H // P         # row chunks per image (and col chunks: W//P)
NB = 3               # boxes per image
F32 = mybir.dt.float32

_PROG = None


def _build_program():
    nc = bacc.Bacc(
        "TRN2",
        target_bir_lowering=False,
        debug=False,
        num_devices=N_CORES,
    )
    o_d = nc.dram_tensor("o", [NIMG, H, W], F32, kind="ExternalInput").ap()
    d_d = nc.dram_tensor("d", [NIMG, H, W], F32, kind="ExternalInput").ap()
    # packed masks per image: cols 0:NCH*NB row mask [y%128, (y//128, j)],
    # cols NCH*NB:2*NCH*NB col mask [x%128, (x//128, j)]
    msk_d = nc.dram_tensor(
        "msk", [NIMG, P, 2 * NCH * NB], F32, kind="ExternalInput"
    ).ap()
    # columns: img0..img2 as 2 halves each, then img3 as 3 quarters + 2
    # eighths; first NCOL are sum(diff) partials, next NCOL are sum(diff^2)
    # partials, then 48 box partials (row 0 only: img-major (img, cx, j))
    NCOL = 2 * (NIMG - 1) + NCH + 1
    NBOXCOL = NIMG * NCH * NB
    acc_d = nc.dram_tensor(
        "acc", [P, 2 * NCOL + NBOXCOL], F32, kind="ExternalOutput"
    ).ap()

    # DRAM views: image rows split as y = c*128 + p  ->  [img, p, c, x]
    o_r = o_d.rearrange("n (c p) x -> n p c x", p=P)
    d_r = d_d.rearrange("n (c p) x -> n p c x", p=P)

    with tile.TileContext(nc) as tc, ExitStack() as ctx:
        io_pool = ctx.enter_context(tc.tile_pool(name="io", bufs=2))
        qio_pool = ctx.enter_context(tc.tile_pool(name="qio", bufs=1))
        mask_pool = ctx.enter_context(tc.tile_pool(name="mask", bufs=2))
        work_pool = ctx.enter_context(tc.tile_pool(name="work", bufs=2))
        psum_pool = ctx.enter_context(tc.tile_pool(name="psum", bufs=2, space="PSUM"))
        acc_pool = ctx.enter_context(tc.tile_pool(name="acc", bufs=1))

        acc = acc_pool.tile([P, 2 * NCOL + NBOXCOL], F32)
        nc.vector.memset(acc[:], 0.0)
        ones_t = acc_pool.tile([P, 1], F32)
        nc.vector.memset(ones_t[:], 1.0)

        def box_work(img, o_chunks, msk_t):
            """o_chunks: list of (tile, free-index) giving [128, 512] y-chunk APs."""
            ps = psum_pool.tile([P, NCH * NB], F32, tag="ps")
            for cx in range(NCH):
                for cy in range(NCH):
                    t, idx = o_chunks[cy]
                    nc.tensor.matmul(
                        ps[:, cx * NB : (cx + 1) * NB],
                        lhsT=t[:, idx, cx * P : (cx + 1) * P],
                        rhs=msk_t[:, cy * NB : (cy + 1) * NB],
                        start=(cy == 0),
                        stop=(cy == NCH - 1),
                    )
            masked_t = work_pool.tile([P, NCH * NB], F32, tag="masked")
            nc.vector.tensor_tensor(
                out=masked_t[:],
                in0=ps[:],
                in1=msk_t[:, NCH * NB : 2 * NCH * NB],
                op=mybir.AluOpType.mult,
            )
            ps2 = psum_pool.tile([1, NCH * NB], F32, tag="ps2")
            nc.tensor.matmul(
                ps2[:], lhsT=ones_t[:], rhs=masked_t[:], start=True, stop=True
            )
            col0 = 2 * NCOL + img * NCH * NB
            nc.vector.tensor_copy(acc[0:1, col0 : col0 + NCH * NB], ps2[:])

        def diff_work(o_ap, d_ap, col, square_on_dve=False, tag=""):
            """stt diff + square over one chunk, accumulating into column col.

            The square runs on ACT by default (hides under DMA); for the tail
            chunks it runs on DVE so the critical chain stays on one engine.
            """
            diff_t = work_pool.tile(
                list(o_ap.shape), F32, tag="diff" + tag, bufs=5 if tag else None
            )
            nc.vector.scalar_tensor_tensor(
                out=diff_t[:],
                in0=o_ap,
                scalar=0.0,
                in1=d_ap,
                op0=mybir.AluOpType.bypass,
                op1=mybir.AluOpType.subtract,
                accum_out=acc[:, col : col + 1],
            )
            sq_t = work_pool.tile(
                list(o_ap.shape), F32, tag="sq" + tag, bufs=5 if tag else None
            )
            if square_on_dve:
                nc.vector.scalar_tensor_tensor(
                    out=sq_t[:],
                    in0=diff_t[:],
                    scalar=0.0,
                    in1=diff_t[:],
                    op0=mybir.AluOpType.bypass,
                    op1=mybir.AluOpType.mult,
                    accum_out=acc[:, NCOL + col : NCOL + col + 1],
                )
            else:
                nc.scalar.activation(
                    sq_t[:],
                    diff_t[:],
                    mybir.ActivationFunctionType.Square,
                    accum_out=acc[:, NCOL + col : NCOL + col + 1],
                )

        msk_all = mask_pool.tile([P, NIMG, 2 * NCH * NB], F32)

        # images 0..NIMG-2: half-image pipeline (keeps DVE/ACT streaming
        # steadily behind the DMA instead of big 2.2us blocks)
        HC = NCH // 2
        for img in range(NIMG - 1):
            halves = []
            for h in range(2):
                o_t = io_pool.tile([P, HC, W], F32, tag=f"o{h}")
                nc.sync.dma_start(o_t[:], o_r[img, :, h * HC : (h + 1) * HC])
                d_t = io_pool.tile([P, HC, W], F32, tag=f"d{h}")
                nc.sync.dma_start(d_t[:], d_r[img, :, h * HC : (h + 1) * HC])
                if img == 0 and h == 0:
                    # all masks in one small DMA, tucked behind the first pair
                    nc.sync.dma_start(
                        msk_all[:], msk_d.rearrange("n p m -> p n m")
                    )
                diff_work(o_t[:], d_t[:], 2 * img + h)
                halves.append(o_t)
            box_work(
                img,
                [(halves[c // HC], c % HC) for c in range(NCH)],
                msk_all[:, img],
            )

        # last image: quarter-chunks with interleaved o/d DMAs (last quarter as
        # two eighths) so the post-DMA tail is only an eighth-image chain
        img = NIMG - 1
        oq_tiles, chunks = [], []
        for c in range(NCH):
            if c < NCH - 1:
                oq = qio_pool.tile([P, 1, W], F32, tag=f"oq{c}")
                nc.sync.dma_start(oq[:], o_r[img, :, c : c + 1])
                dq = qio_pool.tile([P, 1, W], F32, tag=f"dq{c}")
                nc.sync.dma_start(dq[:], d_r[img, :, c : c + 1])
                oq_tiles.append((oq, 0))
                chunks.append((oq[:], dq[:]))
            else:
                # final quarter as two eighth-image pieces
                oq = qio_pool.tile([P, 1, W], F32, tag=f"oq{c}")
                dq = qio_pool.tile([P, 1, W], F32, tag=f"dq{c}")
                for h in range(2):
                    hs = slice(h * (W // 2), (h + 1) * (W // 2))
                    nc.sync.dma_start(oq[:, 0, hs], o_r[img, :, c, hs])
                    nc.sync.dma_start(dq[:, 0, hs], d_r[img, :, c, hs])
                    chunks.append((oq[:, 0, hs], dq[:, 0, hs]))
                oq_tiles.append((oq, 0))
        for i, (o_ap, d_ap) in enumerate(chunks):
            # the very last chunk squares on DVE: keeps the critical chain on
            # one engine with no cross-engine semaphore hop
            diff_work(
                o_ap,
                d_ap,
                2 * (NIMG - 1) + i,
                square_on_dve=(i == len(chunks) - 1),
                tag="q",
            )
        box_work(img, oq_tiles, msk_all[:, img])

        nc.sync.dma_start(acc_d, acc[:])

    nc.compile()
    return nc


def _get_program():
    global _PROG
    if _PROG is None:
        _PROG = _build_program()
    return _PROG


def _prep_inputs(output, density_map, bboxes):
    o = np.ascontiguousarray(np.asarray(output, dtype=np.float32).reshape(B, H, W))
    dm = np.ascontiguousarray(
        np.asarray(density_map, dtype=np.float32).reshape(B, H, W)
    )
    bb = np.clip(np.asarray(bboxes).astype(np.int64), 0, W).astype(np.int32)
    x1, y1, x2, y2 = bb[..., 0], bb[..., 1], bb[..., 2], bb[..., 3]
    x2 = np.maximum(x2, x1)
    y2 = np.maximum(y2, y1)

    ar = np.arange(H, dtype=np.int32)
    # rm[b, y, j] = 1 if y1 <= y < y2, laid out as [b, y%128, (y//128, j)]
    rm = (
        (ar[None, :, None] >= y1[:, None, :]) & (ar[None, :, None] < y2[:, None, :])
    ).astype(np.float32)
    rm = rm.reshape(B, NCH, P, NB).transpose(0, 2, 1, 3).reshape(B, P, NCH * NB)
    # cm[b, j, x] = 1 if x1 <= x < x2, laid out as [b, x%128, (x//128, j)]
    cm = (
        (ar[None, None, :] >= x1[:, :, None]) & (ar[None, None, :] < x2[:, :, None])
    ).astype(np.float32)
    cm = cm.reshape(B, NB, NCH, P).transpose(0, 3, 2, 1).reshape(B, P, NCH * NB)
    msk = np.ascontiguousarray(np.concatenate([rm, cm], axis=2))  # [B, P, 24]
    return o, dm, msk


def kernel(output, density_map, bboxes, num_objects):
    o, dm, msk = _prep_inputs(output, density_map, bboxes)

    nc = _get_program()
    in_maps = [
        {
            "o": o[i * NIMG : (i + 1) * NIMG],
            "d": dm[i * NIMG : (i + 1) * NIMG],
            "msk": msk[i * NIMG : (i + 1) * NIMG],
        }
        for i in range(N_CORES)
    ]
    res = run_bass_kernel_spmd(nc, in_maps, core_ids=list(range(N_CORES)))

    NCOL = 2 * (NIMG - 1) + NCH + 1

    def _per_img(cols):
        # columns: img0..img2 as 2 halves each, img3 as its remaining chunks
        firsts = [cols[2 * i] + cols[2 * i + 1] for i in range(NIMG - 1)]
        return np.array(firsts + [cols[2 * (NIMG - 1) :].sum()])

    per_img_d = np.concatenate(
        [
            _per_img(r["acc"][:, :NCOL].sum(axis=0, dtype=np.float64))
            for r in res.results
        ]
    )  # [B] sum(o - d) per image
    sq_total = float(
        sum(r["acc"][:, NCOL : 2 * NCOL].sum(dtype=np.float64) for r in res.results)
    )  # sum((o - d)^2)
    # acc[0, 2*NCOL + (img, cx, j)] -> sum over cx -> [NIMG, NB], image-major
    box_sums = np.concatenate(
        [
            r["acc"][0, 2 * NCOL :]
            .reshape(NIMG, NCH, NB)
            .sum(axis=1, dtype=np.float64)
            .reshape(-1)
            for r in res.results
        ]
    )  # [B*NB]

    dmap_loss = sq_total / float(num_objects)
    count_loss = float(np.mean(per_img_d**2))
    min_count = float(np.maximum(0.0, 1.0 - box_sums).sum())
    return np.array([dmap_loss, count_loss, min_count], dtype=np.float32)

